# revision 12
# baseline (speedup 1.0000x reference)
"""ContextualConv2d Trainium2 kernel (v2: kh-packed K=96 col-tiled fp16).

Problem: grouped 3x3 conv (N=32, 128ci -> 256co, groups=4, 56x56, pad 1)
plus per-(batch,channel) context bias: out = conv(x, w) + (c @ cwT)[n,co]
+ bias[co].

Sharding (8 cores): core = (group-pair gp in {0,1}) x (batch quarter q in
{0..3}). Each core computes 8 images x 128 out-channels (2 groups of 64).

Per-core compute scheme:
  - x is uploaded in a host-packed "3-band" layout: partition p = kh*32+ci
    holds padded-x rows shifted by kh. This lifts the kh taps into the
    matmul contraction dim: one matmul per kw contracts (ci=32) x (kh=3)
    = K=96, accumulating 3 matmuls per output tile instead of 9.
  - M=64 (one group's out-channels); the two groups run as 2x column
    tiles of the PE array (tile_position (0,0) and (0,64)), which the PE
    can stream concurrently (independent weight cols + rhs xbuses).
  - fp16 operands (fp32 PSUM accumulate) halve DMA and SBUF traffic;
    output is written fp16 and widened to fp32 on the host.
  - epilogue fuses the context/bias add (per-partition scalar) with the
    PSUM->SBUF copy, alternating DVE/ACT engines per half-tile.
"""

import numpy as np

from concourse import bass, mybir, tile
from concourse.vector_clock import ScopedClock
from concourse.bass_utils import run_bass_kernel_spmd

N, CIN, H, W = 32, 128, 56, 56
COUT, KH, KW = 256, 3, 3
GROUPS = 4
CDIM = 64
WP = W + 2            # padded width (58)
HP = H + 2            # padded height (58)
ROWS = 8              # output rows per n-tile
NT = H // ROWS        # 7 n-tiles per image
NFREE = ROWS * W      # 448 <= 512 fp32 PSUM bank limit
N_CORES = 8
IMGS = N // 4         # 8 images per core
CO = COUT // 2        # 128 output channels per core (2 groups)
KPACK = KH * 32       # 96 contraction rows (kh x ci)

# Hybrid split: images at these local indices run the unreplicated
# "row-halves" path (x DMA'd once, 9-tap K=64 block-diag matmuls, PE-heavy);
# the rest run the banded K=96 path (3x-replicated x, PE-light). The split
# balances per-core HBM bytes against PE columns.
ROWH_PAIRS = ((0, 1), (4, 5))
BAND_IMGS = (2, 3, 6, 7)


class _TC(tile.TileContext):
    """This container's walrus accepts only one sem wait on a Drain
    (CTRL) instruction; TileContext's tail drain aggregates one wait per
    outstanding semaphore. Split them across sequential drains."""

    def _drain_and_barrier(self, tick_clock, wait_clock):
        drain_inst = self.nc.sync.drain()
        wait_clock.add_sem_waits(
            drain_inst.ins, ScopedClock({None: tick_clock.global_clock})
        )
        si = drain_inst.ins.sync_info
        if si is not None and len(si.on_wait) > 1:
            waits = list(si.on_wait)
            si.on_wait.clear()
            si.on_wait.append(waits[0])
            for w in waits[1:]:
                d2 = self.nc.sync.drain()
                d2.ins.sync_info = mybir.SyncInfo(on_wait=[w], on_update=[])
        self.nc.all_engine_barrier()
        assert self.sems is not None
        popped = self.nc._tile_sem_poison_stack.pop()
        assert popped is self._sem_poison
        self.nc.clear_and_free_semaphores(list(self.sems.allocated().values()))
        self.nc.all_engine_barrier()


_ws_ctr = [0]


def _split_waits(nc):
    """Walrus here caps sem waits at one per instruction; hoist extras
    onto injected same-engine NoOps placed just before the owner."""
    for fn in nc.m.functions:
        for blk in fn.blocks:
            insts = blk.instructions
            out = []
            changed = False
            for inst in insts:
                si = getattr(inst, "sync_info", None)
                if si is not None and si.on_wait and len(si.on_wait) > 1:
                    waits = list(si.on_wait)
                    for w in waits[:-1]:
                        _ws_ctr[0] += 1
                        out.append(
                            mybir.InstNoOp(
                                name=f"WSNOP-{_ws_ctr[0]}",
                                engine=inst.engine,
                                ins=[],
                                outs=[],
                                sync_info=mybir.SyncInfo(on_wait=[w], on_update=[]),
                                debug=inst.debug,
                            )
                        )
                        changed = True
                    si.on_wait.clear()
                    si.on_wait.append(waits[-1])
                out.append(inst)
            if changed:
                insts.clear()
                insts.extend(out)
    return nc


def build_program(loop_n: int = 0, skip_x: bool = False, skip_y: bool = False,
                  skip_mm: bool = False, skip_epi: bool = False):
    """loop_n > 0 builds a benchmark variant: the conv body repeats
    loop_n times inside a hardware For_i so device time dominates the
    (RPC/transfer-heavy) wall clock. loop_n=0 is the production kernel.
    skip_* build timing-ablation variants (incorrect outputs)."""
    f32 = mybir.dt.float32
    f32r = mybir.dt.float32r
    f16 = mybir.dt.float16
    nc = bass.Bass("TRN2", target_bir_lowering=False, debug=False)
    xs3 = nc.declare_dram_parameter(
        "xs3", [len(BAND_IMGS), KPACK, 2, H, WP], f16, isOutput=False
    )
    xsr = nc.declare_dram_parameter(
        "xsr", [len(ROWH_PAIRS), 128, HP, WP], f16, isOutput=False
    )
    wb = nc.declare_dram_parameter("wb", [KPACK, 6, 64], f16, isOutput=False)
    wbr = nc.declare_dram_parameter("wbr", [128, 9, CO], f16, isOutput=False)
    cwb = nc.declare_dram_parameter("cwb", [CDIM + 1, CO], f32r, isOutput=False)
    cb = nc.declare_dram_parameter("cb", [CDIM + 1, IMGS], f32r, isOutput=False)
    y = nc.declare_dram_parameter("y", [IMGS, CO, H, W], f16, isOutput=True)

    with _TC(nc) as tc:
        with (
            tc.tile_pool(name="wp", bufs=1) as wpool,
            tc.tile_pool(name="xp", bufs=3) as xpool,
            tc.tile_pool(name="op", bufs=4) as opool,
            tc.tile_pool(name="psp", bufs=6, space="PSUM") as pspool,
            tc.tile_pool(name="psc", bufs=1, space="PSUM") as pscpool,
        ):
            wt = wpool.tile([KPACK, 6, 64], f16)
            nc.sync.dma_start(wt[:], wb[:])
            wtr = wpool.tile([128, 9, CO], f16)
            nc.sync.dma_start(wtr[:], wbr[:])
            cwbt = wpool.tile([CDIM + 1, CO], f32r)
            nc.sync.dma_start(cwbt[:], cwb[:])
            cbt = wpool.tile([CDIM + 1, IMGS], f32r)
            nc.sync.dma_start(cbt[:], cb[:])

            # bctx[co, n] = sum_d c_weight[co,d] c[n,d] + bias[co]
            psc = pscpool.tile([CO, IMGS], f32)
            nc.tensor.matmul(psc[:, :], cwbt[:], cbt[:], start=True, stop=True)
            bctx = wpool.tile([CO, IMGS], f32)
            nc.vector.tensor_copy(bctx[:], psc[:, :])

            epi_ctr = [0]

            def epilogue(ps, i, o):
                epi_ctr[0] += 1
                if epi_ctr[0] % 2 == 0:
                    nc.vector.tensor_scalar_add(o, ps[:, :], bctx[:, i : i + 1])
                else:
                    nc.scalar.activation(
                        o, ps[:, :], mybir.ActivationFunctionType.Identity,
                        bias=bctx[:, i : i + 1],
                    )

            def band_img(u, i):
                xdma = nc.sync if u % 2 == 0 else nc.scalar
                ydma = nc.scalar if u % 2 == 0 else nc.sync
                bi = BAND_IMGS.index(i)
                xt = xpool.tile([KPACK, 2, H, WP], f16, name="xb", tag="xb")
                if not skip_x:
                    xdma.dma_start(xt[:], xs3[bi])
                ot = opool.tile([CO, H * W], f16, name=f"ot{i}", tag="ot")
                for t in range(NT):
                    ps = pspool.tile([CO, NFREE], f32, name=f"ps{i}_{t}", tag="ps")
                    if not skip_mm:
                        for kw in range(3):
                            nc.tensor.matmul(
                                ps[0:64, :],
                                wt[:, kw, :],
                                xt[:, 0, t * ROWS : t * ROWS + ROWS, kw : kw + W],
                                start=(kw == 0),
                                stop=(kw == 2),
                                tile_position=(0, 0),
                            )
                            nc.tensor.matmul(
                                ps[64:128, :],
                                wt[:, 3 + kw, :],
                                xt[:, 1, t * ROWS : t * ROWS + ROWS, kw : kw + W],
                                start=(kw == 0),
                                stop=(kw == 2),
                                tile_position=(0, 64),
                            )
                    if not skip_epi:
                        epilogue(ps, i, ot[:, t * NFREE : (t + 1) * NFREE])
                if not skip_y:
                    ydma.dma_start(y[i].rearrange("c h w -> c (h w)"), ot[:])

            def rowh_pair(u, pi):
                ia, ib = ROWH_PAIRS[pi]
                xdma = nc.sync if u % 2 == 0 else nc.scalar
                ydma = nc.scalar if u % 2 == 0 else nc.sync
                xt = xpool.tile([128, HP, WP], f16, name="xr", tag="xr")
                if not skip_x:
                    xdma.dma_start(xt[:], xsr[pi])
                otA = opool.tile([CO, H * W], f16, name=f"otA{pi}", tag="ot")
                otB = opool.tile([CO, H * W], f16, name=f"otB{pi}", tag="ot")
                for t in range(NT):
                    psA = pspool.tile([CO, NFREE], f32, name=f"psA{pi}_{t}", tag="ps")
                    psB = pspool.tile([CO, NFREE], f32, name=f"psB{pi}_{t}", tag="ps")
                    if not skip_mm:
                        for p in range(9):
                            kh, kw = p // 3, p % 3
                            h0 = t * ROWS + kh
                            nc.tensor.matmul(
                                psA[:, :],
                                wtr[0:64, p, :],
                                xt[0:64, h0 : h0 + ROWS, kw : kw + W],
                                start=(p == 0),
                                stop=(p == 8),
                                tile_position=(0, 0),
                            )
                            nc.tensor.matmul(
                                psB[:, :],
                                wtr[64:128, p, :],
                                xt[64:128, h0 : h0 + ROWS, kw : kw + W],
                                start=(p == 0),
                                stop=(p == 8),
                                tile_position=(64, 0),
                            )
                    if not skip_epi:
                        epilogue(psA, ia, otA[:, t * NFREE : (t + 1) * NFREE])
                        epilogue(psB, ib, otB[:, t * NFREE : (t + 1) * NFREE])
                if not skip_y:
                    ydma.dma_start(y[ia].rearrange("c h w -> c (h w)"), otA[:])
                    ydma.dma_start(y[ib].rearrange("c h w -> c (h w)"), otB[:])

            def conv_body():
                # interleave PE-heavy row-half pairs with DMA-heavy banded
                # images so neither resource idles long
                rowh_pair(0, 0)
                band_img(1, BAND_IMGS[0])
                band_img(2, BAND_IMGS[1])
                rowh_pair(3, 1)
                band_img(4, BAND_IMGS[2])
                band_img(5, BAND_IMGS[3])

            if loop_n > 0:
                with tc.For_i(0, loop_n, 1, hint_engines=(mybir.EngineType.PE,)):
                    conv_body()
            else:
                conv_body()
    _split_waits(nc)
    return nc


_prog_cache = {}


def _get_program():
    if "nc" not in _prog_cache:
        _prog_cache["nc"] = build_program()
    return _prog_cache["nc"]


def _shard_inputs(x, c, weight, bias, c_weight):
    """Build the per-core input dicts (pure layout prep, no math)."""
    xpad = np.zeros((N, CIN, HP, WP), np.float16)
    xpad[:, :, 1 : H + 1, 1 : W + 1] = x.astype(np.float16)

    w16 = weight.astype(np.float16)  # [256, 32, 3, 3]
    wbs = []
    wbrs = []
    cwbs = []
    for gp in range(2):
        wsl = w16[CO * gp : CO * gp + CO]             # [128, 32, 3, 3]
        # banded path: wb[kh*32+ci, g*3+kw, co] = wsl[64g+co, ci, kh, kw]
        blk = wsl.reshape(2, 64, 32, 3, 3).transpose(3, 2, 0, 4, 1)
        wbs.append(np.ascontiguousarray(blk.reshape(KPACK, 6, 64)))

        # row-half path: block-diagonal position-major weights
        # wbr[a*64 + ci, p, co] nonzero iff ci//32 == co//64 (per 64-block)
        blkr = np.zeros((64, 9, CO), np.float16)
        for g in range(2):
            cosl = wsl[64 * g : 64 * g + 64]          # [64, 32, 3, 3]
            blkr[32 * g : 32 * g + 32, :, 64 * g : 64 * g + 64] = (
                cosl.reshape(64, 32, 9).transpose(1, 2, 0)
            )
        wbrs.append(np.concatenate([blkr, blkr], axis=0))  # [128, 9, 128]

        cwbv = np.empty((CDIM + 1, CO), np.float32)
        cwbv[:CDIM] = c_weight[CO * gp : CO * gp + CO].T
        cwbv[CDIM] = bias[CO * gp : CO * gp + CO]
        cwbs.append(cwbv)

    # banded x: xs3[i, kh*32+ci, g, r, c] = xpad[img, 64gp+32g+ci, r+kh+1-1...]
    xs3s = []
    for gp in range(2):
        sub = xpad[:, 64 * gp : 64 * gp + 64]          # [n, 64, 58, 58]
        sub = sub.reshape(N, 2, 32, HP, WP)            # [n, g, ci, hp, wp]
        bands = np.stack(
            [sub[:, :, :, kh : kh + H, :] for kh in range(KH)], axis=1
        )                                              # [n, kh, g, ci, 56, 58]
        xs3s.append(np.ascontiguousarray(
            bands.transpose(0, 1, 3, 2, 4, 5).reshape(N, KPACK, 2, H, WP)
        ))

    in_maps = []
    for core in range(N_CORES):
        gp, q = divmod(core, 4)
        imgsl = slice(IMGS * q, IMGS * q + IMGS)
        cbv = np.empty((CDIM + 1, IMGS), np.float32)
        cbv[:CDIM] = c[imgsl].T
        cbv[CDIM] = 1.0

        band_globals = [IMGS * q + i for i in BAND_IMGS]
        xs3v = np.ascontiguousarray(xs3s[gp][band_globals])

        xsrv = np.empty((len(ROWH_PAIRS), 128, HP, WP), np.float16)
        for pi, (ia, ib) in enumerate(ROWH_PAIRS):
            xsrv[pi, 0:64] = xpad[IMGS * q + ia, 64 * gp : 64 * gp + 64]
            xsrv[pi, 64:128] = xpad[IMGS * q + ib, 64 * gp : 64 * gp + 64]

        in_maps.append(
            {
                "xs3": xs3v,
                "xsr": np.ascontiguousarray(xsrv),
                "wb": wbs[gp],
                "wbr": wbrs[gp],
                "cwb": cwbs[gp],
                "cb": cbv,
            }
        )
    return in_maps


def kernel(x, c, weight, bias, c_weight):
    x = np.asarray(x, np.float32)
    c = np.asarray(c, np.float32)
    weight = np.asarray(weight, np.float32)
    bias = np.asarray(bias, np.float32)
    c_weight = np.asarray(c_weight, np.float32)

    nc = _get_program()
    in_maps = _shard_inputs(x, c, weight, bias, c_weight)
    res = run_bass_kernel_spmd(nc, in_maps, list(range(N_CORES)), trace=False)

    out = np.empty((N, COUT, H, W), np.float32)
    for core in range(N_CORES):
        gp, q = divmod(core, 4)
        out[IMGS * q : IMGS * q + IMGS, CO * gp : CO * gp + CO] = (
            res.results[core]["y"].astype(np.float32)
        )
    return out


# revision 31
# speedup vs baseline: 1.1163x; 1.1163x over previous
"""ContextualConv2d Trainium2 kernel (v2: kh-packed K=96 col-tiled fp16).

Problem: grouped 3x3 conv (N=32, 128ci -> 256co, groups=4, 56x56, pad 1)
plus per-(batch,channel) context bias: out = conv(x, w) + (c @ cwT)[n,co]
+ bias[co].

Sharding (8 cores): core = (group-pair gp in {0,1}) x (batch quarter q in
{0..3}). Each core computes 8 images x 128 out-channels (2 groups of 64).

Per-core compute scheme:
  - x is uploaded in a host-packed "3-band" layout: partition p = kh*32+ci
    holds padded-x rows shifted by kh. This lifts the kh taps into the
    matmul contraction dim: one matmul per kw contracts (ci=32) x (kh=3)
    = K=96, accumulating 3 matmuls per output tile instead of 9.
  - M=64 (one group's out-channels); the two groups run as 2x column
    tiles of the PE array (tile_position (0,0) and (0,64)), which the PE
    can stream concurrently (independent weight cols + rhs xbuses).
  - fp16 operands (fp32 PSUM accumulate) halve DMA and SBUF traffic;
    output is written fp16 and widened to fp32 on the host.
  - epilogue fuses the context/bias add (per-partition scalar) with the
    PSUM->SBUF copy, alternating DVE/ACT engines per half-tile.
"""

import numpy as np

from concourse import bass, mybir, tile
from concourse.vector_clock import ScopedClock
from concourse.bass_utils import run_bass_kernel_spmd

N, CIN, H, W = 32, 128, 56, 56
COUT, KH, KW = 256, 3, 3
GROUPS = 4
CDIM = 64
WP = W + 2            # padded width (58)
HP = H + 2            # padded height (58)
ROWS = 8              # output rows per n-tile
NT = H // ROWS        # 7 n-tiles per image
NFREE = ROWS * W      # 448 <= 512 fp32 PSUM bank limit
N_CORES = 8
IMGS = N // 4         # 8 images per core
CO = COUT // 2        # 128 output channels per core (2 groups)
KPACK = KH * 32       # 96 contraction rows (kh x ci)

# Hybrid split: image pairs in ROWH_PAIRS run the unreplicated
# "row-halves" path (x DMA'd once, 9-tap K=64 block-diag matmuls,
# PE-heavy); images in BAND_IMGS run the banded K=96 path (3x-replicated
# x, PE-light). HBM holds BOTH layouts for all 8 images; only the
# assigned layout is ever transferred. The row-half path measured
# serial-PE in-kernel (no 2x tile overlap), so production is all-banded.
ROWH_PAIRS = ()
BAND_IMGS = (0, 1, 2, 3, 4, 5, 6, 7)


class _TC(tile.TileContext):
    """This container's walrus accepts only one sem wait on a Drain
    (CTRL) instruction; TileContext's tail drain aggregates one wait per
    outstanding semaphore. Split them across sequential drains."""

    def _drain_and_barrier(self, tick_clock, wait_clock):
        drain_inst = self.nc.sync.drain()
        wait_clock.add_sem_waits(
            drain_inst.ins, ScopedClock({None: tick_clock.global_clock})
        )
        si = drain_inst.ins.sync_info
        if si is not None and len(si.on_wait) > 1:
            waits = list(si.on_wait)
            si.on_wait.clear()
            si.on_wait.append(waits[0])
            for w in waits[1:]:
                d2 = self.nc.sync.drain()
                d2.ins.sync_info = mybir.SyncInfo(on_wait=[w], on_update=[])
        self.nc.all_engine_barrier()
        assert self.sems is not None
        popped = self.nc._tile_sem_poison_stack.pop()
        assert popped is self._sem_poison
        self.nc.clear_and_free_semaphores(list(self.sems.allocated().values()))
        self.nc.all_engine_barrier()


_ws_ctr = [0]


def _split_waits(nc):
    """Walrus here caps sem waits at one per instruction; hoist extras
    onto injected same-engine NoOps placed just before the owner."""
    for fn in nc.m.functions:
        for blk in fn.blocks:
            insts = blk.instructions
            out = []
            changed = False
            for inst in insts:
                si = getattr(inst, "sync_info", None)
                if si is not None and si.on_wait and len(si.on_wait) > 1:
                    waits = list(si.on_wait)
                    for w in waits[:-1]:
                        _ws_ctr[0] += 1
                        out.append(
                            mybir.InstNoOp(
                                name=f"WSNOP-{_ws_ctr[0]}",
                                engine=inst.engine,
                                ins=[],
                                outs=[],
                                sync_info=mybir.SyncInfo(on_wait=[w], on_update=[]),
                                debug=inst.debug,
                            )
                        )
                        changed = True
                    si.on_wait.clear()
                    si.on_wait.append(waits[-1])
                out.append(inst)
            if changed:
                insts.clear()
                insts.extend(out)
    return nc


def build_program(loop_n: int = 0, skip_x: bool = False, skip_y: bool = False,
                  skip_mm: bool = False, skip_epi: bool = False,
                  pairs: tuple = None, bands: tuple = None,
                  group_modes: bool = False):
    """loop_n > 0 builds a benchmark variant: the conv body repeats
    loop_n times inside a hardware For_i so device time dominates the
    (RPC/transfer-heavy) wall clock. loop_n=0 is the production kernel.
    skip_* build timing-ablation variants (incorrect outputs).
    pairs/bands override the hybrid split (timing experiments; the DRAM
    params keep the default shapes so _shard_inputs stays valid)."""
    if pairs is None:
        pairs = ROWH_PAIRS
    if bands is None:
        bands = BAND_IMGS
    f32 = mybir.dt.float32
    f32r = mybir.dt.float32r
    f16 = mybir.dt.float16
    nc = bass.Bass("TRN2", target_bir_lowering=False, debug=False)
    xs3 = nc.declare_dram_parameter(
        "xs3", [IMGS, KPACK, 2, H, WP], f16, isOutput=False
    )
    xsr = nc.declare_dram_parameter(
        "xsr", [IMGS // 2, 128, HP, WP], f16, isOutput=False
    )
    wb = nc.declare_dram_parameter("wb", [KPACK, 6, 64], f16, isOutput=False)
    wbr = nc.declare_dram_parameter("wbr", [128, 9, CO], f16, isOutput=False)
    cwb = nc.declare_dram_parameter("cwb", [CDIM + 1, CO], f32r, isOutput=False)
    cb = nc.declare_dram_parameter("cb", [CDIM + 1, IMGS], f32r, isOutput=False)
    y = nc.declare_dram_parameter("y", [IMGS, CO, H, W], f16, isOutput=True)

    with _TC(nc) as tc:
        with (
            tc.tile_pool(name="wp", bufs=1) as wpool,
            tc.tile_pool(name="xp", bufs=3) as xpool,
            tc.tile_pool(name="op", bufs=4) as opool,
            tc.tile_pool(name="psp", bufs=6, space="PSUM") as pspool,
            tc.tile_pool(name="psc", bufs=1, space="PSUM") as pscpool,
        ):
            wt = wpool.tile([KPACK, 6, 64], f16)
            nc.sync.dma_start(wt[:], wb[:])
            wtr = wpool.tile([128, 9, CO], f16)
            nc.sync.dma_start(wtr[:], wbr[:])
            cwbt = wpool.tile([CDIM + 1, CO], f32r)
            nc.sync.dma_start(cwbt[:], cwb[:])
            cbt = wpool.tile([CDIM + 1, IMGS], f32r)
            nc.sync.dma_start(cbt[:], cb[:])

            # bctx[co, n] = sum_d c_weight[co,d] c[n,d] + bias[co]
            psc = pscpool.tile([CO, IMGS], f32)
            nc.tensor.matmul(psc[:, :], cwbt[:], cbt[:], start=True, stop=True)
            bctx = wpool.tile([CO, IMGS], f32)
            nc.vector.tensor_copy(bctx[:], psc[:, :])

            epi_ctr = [0]

            def epilogue(ps, i, o):
                epi_ctr[0] += 1
                if epi_ctr[0] % 2 == 0:
                    nc.vector.tensor_scalar_add(o, ps[:, :], bctx[:, i : i + 1])
                else:
                    nc.scalar.activation(
                        o, ps[:, :], mybir.ActivationFunctionType.Identity,
                        bias=bctx[:, i : i + 1],
                    )

            def band_img(u, i):
                xdma = nc.sync if u % 2 == 0 else nc.scalar
                ydma = nc.scalar if u % 2 == 0 else nc.sync
                xt = xpool.tile([KPACK, 2, H, WP], f16, name="xb", tag="xb")
                if not skip_x:
                    xdma.dma_start(xt[:], xs3[i])
                else:
                    nc.gpsimd.memset(xt[:], 0.5)
                ot = opool.tile([CO, H * W], f16, name=f"ot{i}", tag="ot")
                if skip_epi:
                    nc.gpsimd.memset(ot[:, 0:1], 0.5)
                for t in range(NT):
                    if skip_mm and skip_epi:
                        continue
                    ps = pspool.tile([CO, NFREE], f32, name=f"ps{i}_{t}", tag="ps")
                    if not skip_mm:
                        for kw in range(3):
                            nc.tensor.matmul(
                                ps[0:64, :],
                                wt[:, kw, :],
                                xt[:, 0, t * ROWS : t * ROWS + ROWS, kw : kw + W],
                                start=(kw == 0),
                                stop=(kw == 2),
                                tile_position=(0, 0),
                            )
                            nc.tensor.matmul(
                                ps[64:128, :],
                                wt[:, 3 + kw, :],
                                xt[:, 1, t * ROWS : t * ROWS + ROWS, kw : kw + W],
                                start=(kw == 0),
                                stop=(kw == 2),
                                tile_position=(0, 64),
                            )
                    if not skip_epi:
                        epilogue(ps, i, ot[:, t * NFREE : (t + 1) * NFREE])
                if not skip_y:
                    ydma.dma_start(y[i].rearrange("c h w -> c (h w)"), ot[:])

            def rowh_pair(u, pi):
                ia, ib = pairs[pi]
                assert ib == ia + 1 and ia % 2 == 0
                xdma = nc.sync if u % 2 == 0 else nc.scalar
                ydma = nc.scalar if u % 2 == 0 else nc.sync
                xt = xpool.tile([128, HP, WP], f16, name="xr", tag="xr")
                if not skip_x:
                    xdma.dma_start(xt[:], xsr[ia // 2])
                else:
                    nc.gpsimd.memset(xt[:], 0.5)
                otA = opool.tile([CO, H * W], f16, name=f"otA{pi}", tag="ot")
                otB = opool.tile([CO, H * W], f16, name=f"otB{pi}", tag="ot")
                if skip_epi:
                    nc.gpsimd.memset(otA[:, 0:1], 0.5)
                    nc.gpsimd.memset(otB[:, 0:1], 0.5)
                for t in range(NT):
                    if skip_mm and skip_epi:
                        continue
                    psA = pspool.tile([CO, NFREE], f32, name=f"psA{pi}_{t}", tag="ps")
                    psB = pspool.tile([CO, NFREE], f32, name=f"psB{pi}_{t}", tag="ps")
                    if not skip_mm:
                        for p in range(9):
                            kh, kw = p // 3, p % 3
                            h0 = t * ROWS + kh
                            nc.tensor.matmul(
                                psA[:, :],
                                wtr[0:64, p, :],
                                xt[0:64, h0 : h0 + ROWS, kw : kw + W],
                                start=(p == 0),
                                stop=(p == 8),
                                tile_position=(0, 0),
                            )
                            nc.tensor.matmul(
                                psB[:, :],
                                wtr[64:128, p, :],
                                xt[64:128, h0 : h0 + ROWS, kw : kw + W],
                                start=(p == 0),
                                stop=(p == 8),
                                tile_position=(64, 0),
                            )
                    if not skip_epi:
                        epilogue(psA, ia, otA[:, t * NFREE : (t + 1) * NFREE])
                        epilogue(psB, ib, otB[:, t * NFREE : (t + 1) * NFREE])
                if not skip_y:
                    ydma.dma_start(y[ia].rearrange("c h w -> c (h w)"), otA[:])
                    ydma.dma_start(y[ib].rearrange("c h w -> c (h w)"), otB[:])

            def conv_body():
                # interleave PE-heavy row-half pairs with DMA-heavy banded
                # images so neither resource idles long
                units = []
                npair = len(pairs)
                nband = len(bands)
                if group_modes:
                    # one PE-tiling-mode switch per block: all row-half pairs,
                    # then all banded images
                    units = [("p", pi) for pi in range(npair)] + [
                        ("b", k) for k in range(nband)
                    ]
                else:
                    # spread pairs evenly among banded images
                    for pi in range(npair):
                        units.append(("p", pi))
                        for k in range(pi * nband // npair, (pi + 1) * nband // npair):
                            units.append(("b", k))
                    if npair == 0:
                        units = [("b", k) for k in range(nband)]
                for u, (kind, idx) in enumerate(units):
                    if kind == "p":
                        rowh_pair(u, idx)
                    else:
                        band_img(u, bands[idx])

            if loop_n > 0:
                with tc.For_i(0, loop_n, 1, hint_engines=(mybir.EngineType.PE,)):
                    conv_body()
            else:
                conv_body()
    _split_waits(nc)
    return nc


_prog_cache = {}


def _get_program():
    if "nc" not in _prog_cache:
        _prog_cache["nc"] = build_program()
    return _prog_cache["nc"]


def _shard_inputs(x, c, weight, bias, c_weight):
    """Build the per-core input dicts (pure layout prep, no math)."""
    xpad = np.zeros((N, CIN, HP, WP), np.float16)
    xpad[:, :, 1 : H + 1, 1 : W + 1] = x.astype(np.float16)

    w16 = weight.astype(np.float16)  # [256, 32, 3, 3]
    wbs = []
    wbrs = []
    cwbs = []
    for gp in range(2):
        wsl = w16[CO * gp : CO * gp + CO]             # [128, 32, 3, 3]
        # banded path: wb[kh*32+ci, g*3+kw, co] = wsl[64g+co, ci, kh, kw]
        blk = wsl.reshape(2, 64, 32, 3, 3).transpose(3, 2, 0, 4, 1)
        wbs.append(np.ascontiguousarray(blk.reshape(KPACK, 6, 64)))

        # row-half path: block-diagonal position-major weights
        # wbr[a*64 + ci, p, co] nonzero iff ci//32 == co//64 (per 64-block)
        blkr = np.zeros((64, 9, CO), np.float16)
        for g in range(2):
            cosl = wsl[64 * g : 64 * g + 64]          # [64, 32, 3, 3]
            blkr[32 * g : 32 * g + 32, :, 64 * g : 64 * g + 64] = (
                cosl.reshape(64, 32, 9).transpose(1, 2, 0)
            )
        wbrs.append(np.concatenate([blkr, blkr], axis=0))  # [128, 9, 128]

        cwbv = np.empty((CDIM + 1, CO), np.float32)
        cwbv[:CDIM] = c_weight[CO * gp : CO * gp + CO].T
        cwbv[CDIM] = bias[CO * gp : CO * gp + CO]
        cwbs.append(cwbv)

    # banded x: xs3[i, kh*32+ci, g, r, c] = xpad[img, 64gp+32g+ci, r+kh+1-1...]
    xs3s = []
    for gp in range(2):
        sub = xpad[:, 64 * gp : 64 * gp + 64]          # [n, 64, 58, 58]
        sub = sub.reshape(N, 2, 32, HP, WP)            # [n, g, ci, hp, wp]
        bands = np.stack(
            [sub[:, :, :, kh : kh + H, :] for kh in range(KH)], axis=1
        )                                              # [n, kh, g, ci, 56, 58]
        xs3s.append(np.ascontiguousarray(
            bands.transpose(0, 1, 3, 2, 4, 5).reshape(N, KPACK, 2, H, WP)
        ))

    in_maps = []
    for core in range(N_CORES):
        gp, q = divmod(core, 4)
        imgsl = slice(IMGS * q, IMGS * q + IMGS)
        cbv = np.empty((CDIM + 1, IMGS), np.float32)
        cbv[:CDIM] = c[imgsl].T
        cbv[CDIM] = 1.0

        xs3v = np.ascontiguousarray(xs3s[gp][imgsl])

        xsrv = np.empty((IMGS // 2, 128, HP, WP), np.float16)
        for pi in range(IMGS // 2):
            xsrv[pi, 0:64] = xpad[IMGS * q + 2 * pi, 64 * gp : 64 * gp + 64]
            xsrv[pi, 64:128] = xpad[IMGS * q + 2 * pi + 1, 64 * gp : 64 * gp + 64]

        in_maps.append(
            {
                "xs3": xs3v,
                "xsr": np.ascontiguousarray(xsrv),
                "wb": wbs[gp],
                "wbr": wbrs[gp],
                "cwb": cwbs[gp],
                "cb": cbv,
            }
        )
    return in_maps


def kernel(x, c, weight, bias, c_weight):
    x = np.asarray(x, np.float32)
    c = np.asarray(c, np.float32)
    weight = np.asarray(weight, np.float32)
    bias = np.asarray(bias, np.float32)
    c_weight = np.asarray(c_weight, np.float32)

    nc = _get_program()
    in_maps = _shard_inputs(x, c, weight, bias, c_weight)
    res = run_bass_kernel_spmd(nc, in_maps, list(range(N_CORES)), trace=False)

    out = np.empty((N, COUT, H, W), np.float32)
    for core in range(N_CORES):
        gp, q = divmod(core, 4)
        out[IMGS * q : IMGS * q + IMGS, CO * gp : CO * gp + CO] = (
            res.results[core]["y"].astype(np.float32)
        )
    return out
